# revision 15
# baseline (speedup 1.0000x reference)
"""Trainium2 Bass kernel for nn_AFM (attentional factorization machine).

Mathematical reduction (validated against the reference):
  - softmax over a size-1 axis == 1, so the attention MLP is dead code and
    fAtt = mean(fPI, axis=1).
  - FM identity per (b, m): sum_{i<j} x_i x_j = ((sum_i x_i)^2 - sum_i x_i^2)/2
    with x_i = dense[b,i,m] * v[i,m].
  - Sign-split scaling: with c[m] = Wp[m]/(2P), u[n,m] = v[n,m]*sqrt(|c[m]|)
    and y = d*u, the FM term becomes
      sum_m sign(c[m]) * ((sum_n y)^2 - sum_n y^2).
    Host reorders the m axis so all c>=0 columns come first (K of them);
    then sum_m sign*(sum_n y^2) collapses to TWO plain free-axis sums of y^2
    (one per contiguous sign block) - computed on the otherwise-idle
    Activation engine via Square+accum_out, entirely off the DVE.

Layout: m-major bf16. Host repacks dense to [B, (m=64, n=32)] bf16 (halves
HBM traffic; all DVE tensor ops become 2-byte -> 2x DVE rate) and keeps a
separate f32 [B, 32] copy of dense[:, :, 0] for the numerically dominant
linear term. The FM term is ~1e-3 of the output, so bf16 there is safe.

Sharding: pure data parallel, batch 4096 -> 512 rows on each of 8 cores,
4 tiles of 128 rows.

Per-core engine assignment:
  SYNC: dense loads (tile 0 in four quarter-tile chunks so compute starts
        ~3us earlier; tiles 1-3 in halves), all queued immediately - the
        HWDGE rings are FIFO so completion order = issue order.
  ACT:  param loads on its own HWDGE ring (urep in quarters, first, so the
        first quarter-mul only waits on 128KB of params); per tile two
        Square+accum_out ops over the sign blocks of y (the whole S2 path);
        a warmup square triggers the one-time ACT_TABLE_LOAD early.
  DVE:  per tile: y = d*u (bf16 2x mode; tile 0 as 4 quarter muls, rest as
        one full-tile mul), S1 via 2 pairwise bf16 add-tree levels plus one
        grouped tensor_reduce (axis=X) -> S1 [128, 64] f32, two
        TENSOR_TENSOR_REDUCE ops (+-1, seeded with linear+bias) for the
        signed sum of S1^2, one scalar_tensor_tensor merge per tile, and
        the final [128, 4] output store from the DVE's own DGE ring (saves
        the 900ns DMA-sem hop to SYNC).
"""

import numpy as np

B, N, M = 4096, 32, 64
NM = N * M                  # 2048
HALF = NM // 2              # 1024
QTR = NM // 4               # 512
NCORES = 8
BS = B // NCORES            # 512 rows per core
TILES = BS // 128           # 4 tiles of 128 batch rows per core
P_PAIRS = N * (N - 1) // 2  # 496

_CACHE = {}


def _build_program(K, cstv):
    """K = #m columns with c >= 0 (packed first); cstv = bl+bp baked in."""
    from concourse import bacc, mybir
    from concourse.dve_ops import TENSOR_TENSOR_REDUCE as CTTR

    f32 = mybir.dt.float32
    bf16 = mybir.dt.bfloat16
    Square = mybir.ActivationFunctionType.Square
    sub = mybir.AluOpType.subtract
    add = mybir.AluOpType.add

    nc = bacc.Bacc("TRN2", target_bir_lowering=False, debug=False)
    dense = nc.declare_dram_parameter("dense", [BS, NM], bf16, isOutput=False)
    urep = nc.declare_dram_parameter("urep", [128, NM], bf16, isOutput=False)
    pw = nc.declare_dram_parameter("pw", [128, 2 * TILES * N], f32, isOutput=False)
    out = nc.declare_dram_parameter("out", [128, TILES], f32, isOutput=True)

    sb = lambda name, shape, dt: nc.alloc_sbuf_tensor(name, list(shape), dt)

    urep_t = sb("urep_t", [128, NM], bf16)
    pw_t = sb("pw_t", [128, 2 * TILES * N], f32)
    cst_t = sb("cst_t", [128, 1], f32)
    spw_t = sb("spw_t", [128, TILES * N], f32)
    lin4_t = sb("lin4_t", [128, TILES], f32)
    seed4_t = sb("seed4_t", [128, TILES], f32)
    o2all = sb("o2all", [128, TILES], f32)
    warm_t = sb("warm_t", [128, 1], f32)
    y2j = sb("y2j", [128, NM], bf16)       # ACT square junk output
    junk = sb("junk", [128, M], f32)       # DVE CTTR junk output

    df_t, y_t, l0_t, l1_t, s1_t = [], [], [], [], []
    a1_t, a2_t, sqp_t, sqn_t = [], [], [], []
    for t in range(TILES):
        df_t.append(sb(f"df{t}", [128, NM], bf16))
        y_t.append(sb(f"y{t}", [128, NM], bf16))
        l0_t.append(sb(f"l0_{t}", [128, M * 16], bf16))
        l1_t.append(sb(f"l1_{t}", [128, M * 8], bf16))
        s1_t.append(sb(f"s1_{t}", [128, M], f32))
        a1_t.append(sb(f"a1_{t}", [128, 1], f32))
        a2_t.append(sb(f"a2_{t}", [128, 1], f32))
        sqp_t.append(sb(f"sqp_{t}", [128, 1], f32))
        sqn_t.append(sb(f"sqn_{t}", [128, 1], f32))

    cnt = {"v": 0, "a": 0, "s": 0, "g": 0}
    chains = {}

    def emit(e, ins):
        ins._wait_ge(chains[e], cnt[e]).then_inc(chains[e], 1)
        cnt[e] += 1
        return cnt[e]

    def emit_dma(e, ins, sem, inc, wait=None):
        if wait is not None:
            wsem, wval = wait
            ins._wait_ge(wsem, wval)
        else:
            ins._wait_ge(chains[e], cnt[e])
        ins.then_inc(sem, inc)

    def emit_wait(e, eng, sem, val):
        eng.wait_ge(sem, val).then_inc(chains[e], 1)
        cnt[e] += 1

    # sign blocks as (start, width, sign) over the m axis, skipping empties
    blocks = [(0, K, 1.0), (K, M - K, -1.0)]
    blocks = [b for b in blocks if b[1] > 0]

    # ACT chain values after tile t's squares (scalar block is built after
    # the vector block, so predict its chain; asserted below)
    n_sq = len(blocks)
    ach_sq_done = [2 + n_sq * (t + 1) + (t + 1) for t in range(TILES)]

    mulB_done = [0] * TILES
    o2_done = [0]

    # NOTE on DMA semaphores: a dma_start's +16 completion budget is spread
    # over its descriptors, and descriptors of LATER starts on the shared
    # queues can complete before an earlier start's last descriptor. So a
    # shared semaphore only safely gates at its FULL total; every load that
    # gates compute at an intermediate point gets its own semaphore.
    with (
        nc.Block() as block,
        nc.semaphore("vch") as vch,
        nc.semaphore("ach") as ach,
        nc.semaphore("sch") as sch,
        nc.semaphore("ld0") as ld0,
        nc.semaphore("ld1") as ld1,
        nc.semaphore("ld23") as ld23,
        nc.semaphore("prm") as prm,
        nc.semaphore("prmU") as prmU,
        nc.semaphore("sts") as sts,
    ):
        chains.update(v=vch, a=ach, s=sch, g=sch)

        @block.vector
        def _(dve):
            def mul_range(t, lo, hi):
                return emit("v", dve.tensor_mul(
                    y_t[t].ap()[:, lo:hi], df_t[t].ap()[:, lo:hi],
                    urep_t.ap()[:, lo:hi],
                ))

            def tree(t):
                # n halves 32->16->8 inside each m group (bf16 2x mode),
                # then one grouped reduce [p, 64, 8] -> [p, 64] f32
                src = y_t[t].ap().rearrange("p (m n) -> p m n", m=M)
                d0 = l0_t[t].ap().rearrange("p (m n) -> p m n", m=M)
                emit("v", dve.tensor_add(d0, src[:, :, 0:16], src[:, :, 16:32]))
                d1 = l1_t[t].ap().rearrange("p (m n) -> p m n", m=M)
                emit("v", dve.tensor_add(d1, d0[:, :, 0:8], d0[:, :, 8:16]))
                emit("v", dve.tensor_reduce(
                    s1_t[t].ap(), d1, axis=mybir.AxisListType.X, op=add,
                ))

            def cttrs(t):
                seed = seed4_t.ap()[:, t : t + 1]
                accs = (a1_t[t], a2_t[t])
                for i, (m0, mw, sg) in enumerate(blocks):
                    sl = s1_t[t].ap()[:, m0 : m0 + mw]
                    emit("v", dve._custom_dve(
                        CTTR, out=junk.ap()[:, 0:mw], in0=sl, in1=sl,
                        s0=seed, s1=sg, accum_out=accs[i].ap(),
                    ))
                    seed = accs[i].ap()
                return seed  # [p,1] = seed4 + sum_pos S1^2 - sum_neg S1^2

            def merge(t, a_last):
                emit_wait("v", dve, ach, ach_sq_done[t])
                o2col = o2all.ap()[:, t : t + 1]
                if len(blocks) == 2:
                    # o2 = (sqn - sqp) + a_last
                    emit("v", dve.scalar_tensor_tensor(
                        out=o2col, in0=sqn_t[t].ap(), scalar=sqp_t[t].ap(),
                        in1=a_last, op0=sub, op1=add,
                    ))
                elif blocks[0][2] > 0:  # all positive: o2 = a_last - sqp
                    emit("v", dve.tensor_sub(o2col, a_last, sqp_t[t].ap()))
                else:                   # all negative: o2 = a_last + sqn
                    emit("v", dve.tensor_add(o2col, a_last, sqn_t[t].ap()))

            TN = TILES * N
            # bias constant baked at build time: no DMA, just a memset
            emit("v", dve.memset(cst_t.ap(), cstv))
            # linear term for all 4 tiles runs during the dead DMA wait:
            # lin4[p,t] = sum_n spd[p,t,n]*Wl[n]  (small param lands first)
            emit_wait("v", dve, prm, 16)
            emit("v", dve.tensor_mul(
                spw_t.ap(), pw_t.ap()[:, 0:TN], pw_t.ap()[:, TN : 2 * TN]))
            emit("v", dve.tensor_reduce(
                lin4_t.ap(),
                spw_t.ap().rearrange("p (t n) -> p t n", t=TILES),
                axis=mybir.AxisListType.X, op=add,
            ))
            emit("v", dve.tensor_scalar_add(seed4_t.ap(), lin4_t.ap(), cst_t.ap()))

            a_last = [None] * TILES
            emit_wait("v", dve, prmU, 16)
            emit_wait("v", dve, ld0, 16)
            mulB_done[0] = mul_range(0, 0, NM)
            tree(0)
            a_last[0] = cttrs(0)
            ldsem = [None, ld1, ld23, ld23]
            ldval = [None, 16, 16, 32]
            for t in range(1, TILES):
                emit_wait("v", dve, ldsem[t], ldval[t])
                mulB_done[t] = mul_range(t, 0, NM)
                tree(t)
                a_last[t] = cttrs(t)
                merge(t - 1, a_last[t - 1])
            merge(TILES - 1, a_last[TILES - 1])
            o2_done[0] = cnt["v"]

        @block.scalar
        def _(act):
            # param loads ride the Activation HWDGE ring, issuing in
            # parallel with the dense loads on the SP ring
            emit_wait("a", act, prm, 16)
            # warmup: trigger the one-time ACT_TABLE_LOAD during the DMA lead-in
            emit("a", act.square(warm_t.ap(), cst_t.ap()))
            for t in range(TILES):
                emit_wait("a", act, vch, mulB_done[t])
                accs = (sqp_t[t], sqn_t[t]) if blocks[0][2] > 0 else (sqn_t[t],)
                for i, (m0, mw, sg) in enumerate(blocks):
                    lo, hi = m0 * N, (m0 + mw) * N
                    emit("a", act.activation(
                        out=y2j.ap()[:, lo:hi], in_=y_t[t].ap()[:, lo:hi],
                        func=Square, accum_out=accs[i].ap(),
                    ))
            assert cnt["a"] == ach_sq_done[-1], (cnt["a"], ach_sq_done)
            # output store from the ACT ring (idle by now), gated on merge3;
            # engine-to-engine sem hop is ~100ns vs 900ns for DMA-completion
            emit_dma("a", act.dma_start(out=out.ap(), in_=o2all.ap()),
                     sts, 16, wait=(vch, o2_done[0]))

        @block.gpsimd
        def _(gp):
            # loads issued from GpSimd (idle engine, 25ns/issue vs 565 on SP)
            emit_dma("g", gp.dma_start(out=pw_t.ap(), in_=pw.ap()), prm, 16)
            emit_dma("g", gp.dma_start(out=urep_t.ap(), in_=urep.ap()), prmU, 16)
            emit_dma("g", gp.dma_start(
                out=df_t[0].ap(), in_=dense.ap()[0:128, :]), ld0, 16)
            emit_dma("g", gp.dma_start(
                out=df_t[1].ap(), in_=dense.ap()[128:256, :]), ld1, 16)
            emit_dma("g", gp.dma_start(
                out=df_t[2].ap(), in_=dense.ap()[256:384, :]), ld23, 16)
            emit_dma("g", gp.dma_start(
                out=df_t[3].ap(), in_=dense.ap()[384:512, :]), ld23, 16)

        @block.sync
        def _(sync):
            sync.wait_ge(sts, 16)

    nc.compile()
    return nc


def _get_program(key):
    if key not in _CACHE:
        _CACHE[key] = _build_program(*key)
    return _CACHE[key]


def _host_prep(inputs):
    import ml_dtypes

    dense = np.asarray(inputs["dense"], dtype=np.float32)  # [B, N, M]
    v = np.asarray(inputs["v"], dtype=np.float32)          # [N, M]
    Wl = np.asarray(inputs["Wl"], dtype=np.float32).reshape(N)
    Wp = np.asarray(inputs["Wp"], dtype=np.float32).reshape(M)
    bl = float(np.asarray(inputs["bl"], dtype=np.float32).reshape(-1)[0])
    bp = float(np.asarray(inputs["bp"], dtype=np.float32).reshape(-1)[0])

    c = (Wp / (2.0 * P_PAIRS)).astype(np.float32)
    pos = np.where(c >= 0)[0]
    neg = np.where(c < 0)[0]
    idx = np.concatenate([pos, neg])
    K = int(len(pos))

    # m-major, sign-sorted, sqrt|c|-scaled replica of v -> u [64, 32]
    u = (v * np.sqrt(np.abs(c))[None, :]).T[idx]               # [M, N]
    urep = np.ascontiguousarray(np.broadcast_to(
        u.reshape(1, NM).astype(ml_dtypes.bfloat16), (128, NM)))

    # dense repacked m-major + sign-sorted: [B, (m, n)] bf16
    dmm = np.ascontiguousarray(
        dense.transpose(0, 2, 1)[:, idx, :].reshape(B, NM)
    ).astype(ml_dtypes.bfloat16)

    sparse = np.ascontiguousarray(dense[:, :, 0])              # [B, N] f32
    wlrep4 = np.broadcast_to(np.tile(Wl, TILES)[None, :], (128, TILES * N))
    cstv = float(bl + bp)

    in_maps = []
    for i in range(NCORES):
        spdi = (
            sparse[BS * i : BS * (i + 1)]
            .reshape(TILES, 128, N).transpose(1, 0, 2).reshape(128, TILES * N)
        )
        pwi = np.ascontiguousarray(np.concatenate([spdi, wlrep4], axis=1))
        in_maps.append({
            "dense": dmm[BS * i : BS * (i + 1)],
            "urep": urep,
            "pw": pwi,
        })
    return (K, cstv), in_maps


def _gather(res):
    # out[p, t] holds batch row 128*t + p of the core's shard
    outs = []
    for i in range(NCORES):
        arr = np.asarray(res.results[i]["out"], np.float32)  # [128, TILES]
        outs.append(arr.T.reshape(BS))
    return np.concatenate(outs).reshape(B, 1)


def kernel(**inputs) -> np.ndarray:
    from concourse.bass_utils import run_bass_kernel_spmd

    K, in_maps = _host_prep(inputs)
    nc = _get_program(K)
    res = run_bass_kernel_spmd(nc, in_maps, core_ids=list(range(NCORES)))
    return _gather(res)


# revision 16
# speedup vs baseline: 1.0063x; 1.0063x over previous
"""Trainium2 Bass kernel for nn_AFM (attentional factorization machine).

Mathematical reduction (validated against the reference):
  - softmax over a size-1 axis == 1, so the attention MLP is dead code and
    fAtt = mean(fPI, axis=1).
  - FM identity per (b, m): sum_{i<j} x_i x_j = ((sum_i x_i)^2 - sum_i x_i^2)/2
    with x_i = dense[b,i,m] * v[i,m].
  - Sign-split scaling: with c[m] = Wp[m]/(2P), u[n,m] = v[n,m]*sqrt(|c[m]|)
    and y = d*u, the FM term becomes
      sum_m sign(c[m]) * ((sum_n y)^2 - sum_n y^2).
    Host reorders the m axis so all c>=0 columns come first (K of them);
    then sum_m sign*(sum_n y^2) collapses to TWO plain free-axis sums of y^2
    (one per contiguous sign block) - computed on the otherwise-idle
    Activation engine via Square+accum_out, entirely off the DVE.

Layout: m-major bf16. Host repacks dense to [B, (m=64, n=32)] bf16 (halves
HBM traffic; all DVE tensor ops become 2-byte -> 2x DVE rate) and keeps a
separate f32 [B, 32] copy of dense[:, :, 0] for the numerically dominant
linear term. The FM term is ~1e-3 of the output, so bf16 there is safe.

Sharding: pure data parallel, batch 4096 -> 512 rows on each of 8 cores,
4 tiles of 128 rows.

Per-core engine assignment:
  SYNC: dense loads (tile 0 in four quarter-tile chunks so compute starts
        ~3us earlier; tiles 1-3 in halves), all queued immediately - the
        HWDGE rings are FIFO so completion order = issue order.
  ACT:  param loads on its own HWDGE ring (urep in quarters, first, so the
        first quarter-mul only waits on 128KB of params); per tile two
        Square+accum_out ops over the sign blocks of y (the whole S2 path);
        a warmup square triggers the one-time ACT_TABLE_LOAD early.
  DVE:  per tile: y = d*u (bf16 2x mode; tile 0 as 4 quarter muls, rest as
        one full-tile mul), S1 via 2 pairwise bf16 add-tree levels plus one
        grouped tensor_reduce (axis=X) -> S1 [128, 64] f32, two
        TENSOR_TENSOR_REDUCE ops (+-1, seeded with linear+bias) for the
        signed sum of S1^2, one scalar_tensor_tensor merge per tile, and
        the final [128, 4] output store from the DVE's own DGE ring (saves
        the 900ns DMA-sem hop to SYNC).
"""

import numpy as np

B, N, M = 4096, 32, 64
NM = N * M                  # 2048
HALF = NM // 2              # 1024
QTR = NM // 4               # 512
NCORES = 8
BS = B // NCORES            # 512 rows per core
TILES = BS // 128           # 4 tiles of 128 batch rows per core
P_PAIRS = N * (N - 1) // 2  # 496

_CACHE = {}


def _build_program(K, cstv):
    """K = #m columns with c >= 0 (packed first); cstv = bl+bp baked in."""
    from concourse import bacc, mybir
    from concourse.dve_ops import TENSOR_TENSOR_REDUCE as CTTR

    f32 = mybir.dt.float32
    bf16 = mybir.dt.bfloat16
    Square = mybir.ActivationFunctionType.Square
    sub = mybir.AluOpType.subtract
    add = mybir.AluOpType.add

    nc = bacc.Bacc("TRN2", target_bir_lowering=False, debug=False)
    dense = nc.declare_dram_parameter("dense", [BS, NM], bf16, isOutput=False)
    urep = nc.declare_dram_parameter("urep", [128, NM], bf16, isOutput=False)
    pw = nc.declare_dram_parameter("pw", [128, 2 * TILES * N], f32, isOutput=False)
    out = nc.declare_dram_parameter("out", [128, TILES], f32, isOutput=True)

    sb = lambda name, shape, dt: nc.alloc_sbuf_tensor(name, list(shape), dt)

    urep_t = sb("urep_t", [128, NM], bf16)
    pw_t = sb("pw_t", [128, 2 * TILES * N], f32)
    cst_t = sb("cst_t", [128, 1], f32)
    spw_t = sb("spw_t", [128, TILES * N], f32)
    lin4_t = sb("lin4_t", [128, TILES], f32)
    seed4_t = sb("seed4_t", [128, TILES], f32)
    o2all = sb("o2all", [128, TILES], f32)
    warm_t = sb("warm_t", [128, 1], f32)
    y2j = sb("y2j", [128, NM], bf16)       # ACT square junk output
    junk = sb("junk", [128, M], f32)       # DVE CTTR junk output

    df_t, y_t, l0_t, l1_t, s1_t = [], [], [], [], []
    a1_t, a2_t, sqp_t, sqn_t = [], [], [], []
    for t in range(TILES):
        df_t.append(sb(f"df{t}", [128, NM], bf16))
        y_t.append(sb(f"y{t}", [128, NM], bf16))
        l0_t.append(sb(f"l0_{t}", [128, M * 16], bf16))
        l1_t.append(sb(f"l1_{t}", [128, M * 8], bf16))
        s1_t.append(sb(f"s1_{t}", [128, M], f32))
        a1_t.append(sb(f"a1_{t}", [128, 1], f32))
        a2_t.append(sb(f"a2_{t}", [128, 1], f32))
        sqp_t.append(sb(f"sqp_{t}", [128, 1], f32))
        sqn_t.append(sb(f"sqn_{t}", [128, 1], f32))

    cnt = {"v": 0, "a": 0, "s": 0, "g": 0}
    chains = {}

    def emit(e, ins):
        ins._wait_ge(chains[e], cnt[e]).then_inc(chains[e], 1)
        cnt[e] += 1
        return cnt[e]

    def emit_dma(e, ins, sem, inc, wait=None):
        if wait is not None:
            wsem, wval = wait
            ins._wait_ge(wsem, wval)
        else:
            ins._wait_ge(chains[e], cnt[e])
        ins.then_inc(sem, inc)

    def emit_wait(e, eng, sem, val):
        eng.wait_ge(sem, val).then_inc(chains[e], 1)
        cnt[e] += 1

    # sign blocks as (start, width, sign) over the m axis, skipping empties
    blocks = [(0, K, 1.0), (K, M - K, -1.0)]
    blocks = [b for b in blocks if b[1] > 0]

    # ACT chain values after tile t's squares (scalar block is built after
    # the vector block, so predict its chain; asserted below)
    n_sq = len(blocks)
    ach_sq_done = [2 + n_sq * (t + 1) + (t + 1) for t in range(TILES)]

    mulB_done = [0] * TILES
    o2_done = [0]

    # NOTE on DMA semaphores: a dma_start's +16 completion budget is spread
    # over its descriptors, and descriptors of LATER starts on the shared
    # queues can complete before an earlier start's last descriptor. So a
    # shared semaphore only safely gates at its FULL total; every load that
    # gates compute at an intermediate point gets its own semaphore.
    with (
        nc.Block() as block,
        nc.semaphore("vch") as vch,
        nc.semaphore("ach") as ach,
        nc.semaphore("sch") as sch,
        nc.semaphore("ld0") as ld0,
        nc.semaphore("ld1") as ld1,
        nc.semaphore("ld23") as ld23,
        nc.semaphore("prm") as prm,
        nc.semaphore("prmU") as prmU,
        nc.semaphore("sts") as sts,
    ):
        chains.update(v=vch, a=ach, s=sch, g=sch)

        @block.vector
        def _(dve):
            def mul_range(t, lo, hi):
                return emit("v", dve.tensor_mul(
                    y_t[t].ap()[:, lo:hi], df_t[t].ap()[:, lo:hi],
                    urep_t.ap()[:, lo:hi],
                ))

            def tree(t):
                # n halves 32->16->8 inside each m group (bf16 2x mode),
                # then one grouped reduce [p, 64, 8] -> [p, 64] f32
                src = y_t[t].ap().rearrange("p (m n) -> p m n", m=M)
                d0 = l0_t[t].ap().rearrange("p (m n) -> p m n", m=M)
                emit("v", dve.tensor_add(d0, src[:, :, 0:16], src[:, :, 16:32]))
                d1 = l1_t[t].ap().rearrange("p (m n) -> p m n", m=M)
                emit("v", dve.tensor_add(d1, d0[:, :, 0:8], d0[:, :, 8:16]))
                emit("v", dve.tensor_reduce(
                    s1_t[t].ap(), d1, axis=mybir.AxisListType.X, op=add,
                ))

            def cttrs(t):
                seed = seed4_t.ap()[:, t : t + 1]
                accs = (a1_t[t], a2_t[t])
                for i, (m0, mw, sg) in enumerate(blocks):
                    sl = s1_t[t].ap()[:, m0 : m0 + mw]
                    emit("v", dve._custom_dve(
                        CTTR, out=junk.ap()[:, 0:mw], in0=sl, in1=sl,
                        s0=seed, s1=sg, accum_out=accs[i].ap(),
                    ))
                    seed = accs[i].ap()
                return seed  # [p,1] = seed4 + sum_pos S1^2 - sum_neg S1^2

            def merge(t, a_last):
                emit_wait("v", dve, ach, ach_sq_done[t])
                o2col = o2all.ap()[:, t : t + 1]
                if len(blocks) == 2:
                    # o2 = (sqn - sqp) + a_last
                    emit("v", dve.scalar_tensor_tensor(
                        out=o2col, in0=sqn_t[t].ap(), scalar=sqp_t[t].ap(),
                        in1=a_last, op0=sub, op1=add,
                    ))
                elif blocks[0][2] > 0:  # all positive: o2 = a_last - sqp
                    emit("v", dve.tensor_sub(o2col, a_last, sqp_t[t].ap()))
                else:                   # all negative: o2 = a_last + sqn
                    emit("v", dve.tensor_add(o2col, a_last, sqn_t[t].ap()))

            TN = TILES * N
            # bias constant baked at build time: no DMA, just a memset
            emit("v", dve.memset(cst_t.ap(), cstv))
            # linear term for all 4 tiles runs during the dead DMA wait:
            # lin4[p,t] = sum_n spd[p,t,n]*Wl[n]  (small param lands first)
            emit_wait("v", dve, prm, 16)
            emit("v", dve.tensor_mul(
                spw_t.ap(), pw_t.ap()[:, 0:TN], pw_t.ap()[:, TN : 2 * TN]))
            emit("v", dve.tensor_reduce(
                lin4_t.ap(),
                spw_t.ap().rearrange("p (t n) -> p t n", t=TILES),
                axis=mybir.AxisListType.X, op=add,
            ))
            emit("v", dve.tensor_scalar_add(seed4_t.ap(), lin4_t.ap(), cst_t.ap()))

            a_last = [None] * TILES
            emit_wait("v", dve, prmU, 16)
            emit_wait("v", dve, ld0, 16)
            mulB_done[0] = mul_range(0, 0, NM)
            tree(0)
            a_last[0] = cttrs(0)
            ldsem = [None, ld1, ld23, ld23]
            ldval = [None, 16, 16, 32]
            for t in range(1, TILES):
                emit_wait("v", dve, ldsem[t], ldval[t])
                mulB_done[t] = mul_range(t, 0, NM)
                tree(t)
                a_last[t] = cttrs(t)
                merge(t - 1, a_last[t - 1])
            merge(TILES - 1, a_last[TILES - 1])
            o2_done[0] = cnt["v"]

        @block.scalar
        def _(act):
            # param loads ride the Activation HWDGE ring, issuing in
            # parallel with the dense loads on the SP ring
            emit_wait("a", act, prm, 16)
            # warmup: trigger the one-time ACT_TABLE_LOAD during the DMA lead-in
            emit("a", act.square(warm_t.ap(), cst_t.ap()))
            for t in range(TILES):
                emit_wait("a", act, vch, mulB_done[t])
                accs = (sqp_t[t], sqn_t[t]) if blocks[0][2] > 0 else (sqn_t[t],)
                for i, (m0, mw, sg) in enumerate(blocks):
                    lo, hi = m0 * N, (m0 + mw) * N
                    emit("a", act.activation(
                        out=y2j.ap()[:, lo:hi], in_=y_t[t].ap()[:, lo:hi],
                        func=Square, accum_out=accs[i].ap(),
                    ))
            assert cnt["a"] == ach_sq_done[-1], (cnt["a"], ach_sq_done)
            # output store from the ACT ring (idle by now), gated on merge3;
            # engine-to-engine sem hop is ~100ns vs 900ns for DMA-completion
            emit_dma("a", act.dma_start(out=out.ap(), in_=o2all.ap()),
                     sts, 16, wait=(vch, o2_done[0]))

        @block.sync
        def _(sync):
            # single ring, few big starts in exact need-order. The DGE
            # needs ~2us to prep a start; back-to-back small starts drain
            # the queues and expose that latency, so keep starts >= 512KB.
            emit_dma("s", sync.dma_start(out=pw_t.ap(), in_=pw.ap()), prm, 16)
            emit_dma("s", sync.dma_start(out=urep_t.ap(), in_=urep.ap()), prmU, 16)
            emit_dma("s", sync.dma_start(
                out=df_t[0].ap(), in_=dense.ap()[0:128, :]), ld0, 16)
            emit_dma("s", sync.dma_start(
                out=df_t[1].ap(), in_=dense.ap()[128:256, :]), ld1, 16)
            emit_dma("s", sync.dma_start(
                out=df_t[2].ap(), in_=dense.ap()[256:384, :]), ld23, 16)
            emit_dma("s", sync.dma_start(
                out=df_t[3].ap(), in_=dense.ap()[384:512, :]), ld23, 16)
            sync.wait_ge(sts, 16)

    nc.compile()
    return nc


def _get_program(key):
    if key not in _CACHE:
        _CACHE[key] = _build_program(*key)
    return _CACHE[key]


def _host_prep(inputs):
    import ml_dtypes

    dense = np.asarray(inputs["dense"], dtype=np.float32)  # [B, N, M]
    v = np.asarray(inputs["v"], dtype=np.float32)          # [N, M]
    Wl = np.asarray(inputs["Wl"], dtype=np.float32).reshape(N)
    Wp = np.asarray(inputs["Wp"], dtype=np.float32).reshape(M)
    bl = float(np.asarray(inputs["bl"], dtype=np.float32).reshape(-1)[0])
    bp = float(np.asarray(inputs["bp"], dtype=np.float32).reshape(-1)[0])

    c = (Wp / (2.0 * P_PAIRS)).astype(np.float32)
    pos = np.where(c >= 0)[0]
    neg = np.where(c < 0)[0]
    idx = np.concatenate([pos, neg])
    K = int(len(pos))

    # m-major, sign-sorted, sqrt|c|-scaled replica of v -> u [64, 32]
    u = (v * np.sqrt(np.abs(c))[None, :]).T[idx]               # [M, N]
    urep = np.ascontiguousarray(np.broadcast_to(
        u.reshape(1, NM).astype(ml_dtypes.bfloat16), (128, NM)))

    # dense repacked m-major + sign-sorted: [B, (m, n)] bf16
    dmm = np.ascontiguousarray(
        dense.transpose(0, 2, 1)[:, idx, :].reshape(B, NM)
    ).astype(ml_dtypes.bfloat16)

    sparse = np.ascontiguousarray(dense[:, :, 0])              # [B, N] f32
    wlrep4 = np.broadcast_to(np.tile(Wl, TILES)[None, :], (128, TILES * N))
    cstv = float(bl + bp)

    in_maps = []
    for i in range(NCORES):
        spdi = (
            sparse[BS * i : BS * (i + 1)]
            .reshape(TILES, 128, N).transpose(1, 0, 2).reshape(128, TILES * N)
        )
        pwi = np.ascontiguousarray(np.concatenate([spdi, wlrep4], axis=1))
        in_maps.append({
            "dense": dmm[BS * i : BS * (i + 1)],
            "urep": urep,
            "pw": pwi,
        })
    return (K, cstv), in_maps


def _gather(res):
    # out[p, t] holds batch row 128*t + p of the core's shard
    outs = []
    for i in range(NCORES):
        arr = np.asarray(res.results[i]["out"], np.float32)  # [128, TILES]
        outs.append(arr.T.reshape(BS))
    return np.concatenate(outs).reshape(B, 1)


def kernel(**inputs) -> np.ndarray:
    from concourse.bass_utils import run_bass_kernel_spmd

    K, in_maps = _host_prep(inputs)
    nc = _get_program(K)
    res = run_bass_kernel_spmd(nc, in_maps, core_ids=list(range(NCORES)))
    return _gather(res)


# revision 18
# speedup vs baseline: 1.0078x; 1.0014x over previous
"""Trainium2 Bass kernel for nn_AFM (attentional factorization machine).

Mathematical reduction (validated against the reference):
  - softmax over a size-1 axis == 1, so the attention MLP is dead code and
    fAtt = mean(fPI, axis=1).
  - FM identity per (b, m): sum_{i<j} x_i x_j = ((sum_i x_i)^2 - sum_i x_i^2)/2
    with x_i = dense[b,i,m] * v[i,m].
  - Sign-split scaling: with c[m] = Wp[m]/(2P), u[n,m] = v[n,m]*sqrt(|c[m]|)
    and y = d*u, the FM term becomes
      sum_m sign(c[m]) * ((sum_n y)^2 - sum_n y^2).
    Host reorders the m axis so all c>=0 columns come first (K of them);
    then sum_m sign*(sum_n y^2) collapses to TWO plain free-axis sums of y^2
    (one per contiguous sign block) - computed on the otherwise-idle
    Activation engine via Square+accum_out, entirely off the DVE.

Layout: m-major bf16. Host repacks dense to [B, (m=64, n=32)] bf16 (halves
HBM traffic; all DVE tensor ops become 2-byte -> 2x DVE rate) and keeps a
separate f32 [B, 32] copy of dense[:, :, 0] for the numerically dominant
linear term. The FM term is ~1e-3 of the output, so bf16 there is safe.

Sharding: pure data parallel, batch 4096 -> 512 rows on each of 8 cores,
4 tiles of 128 rows.

Per-core engine assignment:
  SYNC: dense loads (tile 0 in four quarter-tile chunks so compute starts
        ~3us earlier; tiles 1-3 in halves), all queued immediately - the
        HWDGE rings are FIFO so completion order = issue order.
  ACT:  param loads on its own HWDGE ring (urep in quarters, first, so the
        first quarter-mul only waits on 128KB of params); per tile two
        Square+accum_out ops over the sign blocks of y (the whole S2 path);
        a warmup square triggers the one-time ACT_TABLE_LOAD early.
  DVE:  per tile: y = d*u (bf16 2x mode; tile 0 as 4 quarter muls, rest as
        one full-tile mul), S1 via 2 pairwise bf16 add-tree levels plus one
        grouped tensor_reduce (axis=X) -> S1 [128, 64] f32, two
        TENSOR_TENSOR_REDUCE ops (+-1, seeded with linear+bias) for the
        signed sum of S1^2, one scalar_tensor_tensor merge per tile, and
        the final [128, 4] output store from the DVE's own DGE ring (saves
        the 900ns DMA-sem hop to SYNC).
"""

import numpy as np

B, N, M = 4096, 32, 64
NM = N * M                  # 2048
HALF = NM // 2              # 1024
QTR = NM // 4               # 512
NCORES = 8
BS = B // NCORES            # 512 rows per core
TILES = BS // 128           # 4 tiles of 128 batch rows per core
P_PAIRS = N * (N - 1) // 2  # 496

_CACHE = {}


def _build_program(K, cstv):
    """K = #m columns with c >= 0 (packed first); cstv = bl+bp baked in."""
    from concourse import bacc, mybir
    from concourse.dve_ops import TENSOR_TENSOR_REDUCE as CTTR

    f32 = mybir.dt.float32
    bf16 = mybir.dt.bfloat16
    Square = mybir.ActivationFunctionType.Square
    sub = mybir.AluOpType.subtract
    add = mybir.AluOpType.add

    nc = bacc.Bacc("TRN2", target_bir_lowering=False, debug=False)
    dense = nc.declare_dram_parameter("dense", [BS, NM], bf16, isOutput=False)
    urep = nc.declare_dram_parameter("urep", [128, NM], bf16, isOutput=False)
    pw = nc.declare_dram_parameter("pw", [128, 2 * TILES * N], f32, isOutput=False)
    out = nc.declare_dram_parameter("out", [128, TILES], f32, isOutput=True)

    sb = lambda name, shape, dt: nc.alloc_sbuf_tensor(name, list(shape), dt)

    urep_t = sb("urep_t", [128, NM], bf16)
    pw_t = sb("pw_t", [128, 2 * TILES * N], f32)
    cst_t = sb("cst_t", [128, 1], f32)
    spw_t = sb("spw_t", [128, TILES * N], f32)
    lin4_t = sb("lin4_t", [128, TILES], f32)
    seed4_t = sb("seed4_t", [128, TILES], f32)
    o2all = sb("o2all", [128, TILES], f32)
    warm_t = sb("warm_t", [128, 1], f32)
    y2j = sb("y2j", [128, NM], bf16)       # ACT square junk output
    junk = sb("junk", [128, M], f32)       # DVE CTTR junk output

    df_t, y_t, l0_t, l1_t, s1_t = [], [], [], [], []
    a1_t, a2_t, sqp_t, sqn_t = [], [], [], []
    for t in range(TILES):
        df_t.append(sb(f"df{t}", [128, NM], bf16))
        y_t.append(sb(f"y{t}", [128, NM], bf16))
        l0_t.append(sb(f"l0_{t}", [128, M * 16], bf16))
        l1_t.append(sb(f"l1_{t}", [128, M * 8], bf16))
        s1_t.append(sb(f"s1_{t}", [128, M], f32))
        a1_t.append(sb(f"a1_{t}", [128, 1], f32))
        a2_t.append(sb(f"a2_{t}", [128, 1], f32))
        sqp_t.append(sb(f"sqp_{t}", [128, 1], f32))
        sqn_t.append(sb(f"sqn_{t}", [128, 1], f32))

    cnt = {"v": 0, "a": 0, "s": 0, "g": 0}
    chains = {}

    def emit(e, ins):
        ins._wait_ge(chains[e], cnt[e]).then_inc(chains[e], 1)
        cnt[e] += 1
        return cnt[e]

    def emit_dma(e, ins, sem, inc, wait=None):
        if wait is not None:
            wsem, wval = wait
            ins._wait_ge(wsem, wval)
        else:
            ins._wait_ge(chains[e], cnt[e])
        ins.then_inc(sem, inc)

    def emit_wait(e, eng, sem, val):
        eng.wait_ge(sem, val).then_inc(chains[e], 1)
        cnt[e] += 1

    # sign blocks as (start, width, sign) over the m axis, skipping empties
    blocks = [(0, K, 1.0), (K, M - K, -1.0)]
    blocks = [b for b in blocks if b[1] > 0]

    # ACT chain values after tile t's squares (scalar block is built after
    # the vector block, so predict its chain; asserted below)
    n_sq = len(blocks)
    ach_sq_done = [2 + n_sq * (t + 1) + (t + 1) for t in range(TILES)]

    mulB_done = [0] * TILES
    o2_done = [0]

    # NOTE on DMA semaphores: a dma_start's +16 completion budget is spread
    # over its descriptors, and descriptors of LATER starts on the shared
    # queues can complete before an earlier start's last descriptor. So a
    # shared semaphore only safely gates at its FULL total; every load that
    # gates compute at an intermediate point gets its own semaphore.
    with (
        nc.Block() as block,
        nc.semaphore("vch") as vch,
        nc.semaphore("ach") as ach,
        nc.semaphore("sch") as sch,
        nc.semaphore("ld0") as ld0,
        nc.semaphore("ld1") as ld1,
        nc.semaphore("ld23") as ld23,
        nc.semaphore("prm") as prm,
        nc.semaphore("prmU") as prmU,
        nc.semaphore("sts") as sts,
    ):
        chains.update(v=vch, a=ach, s=sch)

        @block.vector
        def _(dve):
            def mul_range(t, lo, hi):
                return emit("v", dve.tensor_mul(
                    y_t[t].ap()[:, lo:hi], df_t[t].ap()[:, lo:hi],
                    urep_t.ap()[:, lo:hi],
                ))

            def tree(t):
                # n halves 32->16->8 inside each m group (bf16 2x mode),
                # then one grouped reduce [p, 64, 8] -> [p, 64] f32
                src = y_t[t].ap().rearrange("p (m n) -> p m n", m=M)
                d0 = l0_t[t].ap().rearrange("p (m n) -> p m n", m=M)
                emit("v", dve.tensor_add(d0, src[:, :, 0:16], src[:, :, 16:32]))
                d1 = l1_t[t].ap().rearrange("p (m n) -> p m n", m=M)
                emit("v", dve.tensor_add(d1, d0[:, :, 0:8], d0[:, :, 8:16]))
                emit("v", dve.tensor_reduce(
                    s1_t[t].ap(), d1, axis=mybir.AxisListType.X, op=add,
                ))

            def cttrs(t):
                seed = seed4_t.ap()[:, t : t + 1]
                accs = (a1_t[t], a2_t[t])
                for i, (m0, mw, sg) in enumerate(blocks):
                    sl = s1_t[t].ap()[:, m0 : m0 + mw]
                    emit("v", dve._custom_dve(
                        CTTR, out=junk.ap()[:, 0:mw], in0=sl, in1=sl,
                        s0=seed, s1=sg, accum_out=accs[i].ap(),
                    ))
                    seed = accs[i].ap()
                return seed  # [p,1] = seed4 + sum_pos S1^2 - sum_neg S1^2

            def merge(t, a_last):
                emit_wait("v", dve, ach, ach_sq_done[t])
                o2col = o2all.ap()[:, t : t + 1]
                if len(blocks) == 2:
                    # o2 = (sqn - sqp) + a_last
                    emit("v", dve.scalar_tensor_tensor(
                        out=o2col, in0=sqn_t[t].ap(), scalar=sqp_t[t].ap(),
                        in1=a_last, op0=sub, op1=add,
                    ))
                elif blocks[0][2] > 0:  # all positive: o2 = a_last - sqp
                    emit("v", dve.tensor_sub(o2col, a_last, sqp_t[t].ap()))
                else:                   # all negative: o2 = a_last + sqn
                    emit("v", dve.tensor_add(o2col, a_last, sqn_t[t].ap()))

            TN = TILES * N
            # bias constant baked at build time: no DMA, just a memset
            emit("v", dve.memset(cst_t.ap(), cstv))
            # linear term for all 4 tiles runs during the dead DMA wait:
            # lin4[p,t] = sum_n spd[p,t,n]*Wl[n]  (small param lands first)
            emit_wait("v", dve, prm, 16)
            emit("v", dve.tensor_mul(
                spw_t.ap(), pw_t.ap()[:, 0:TN], pw_t.ap()[:, TN : 2 * TN]))
            emit("v", dve.tensor_reduce(
                lin4_t.ap(),
                spw_t.ap().rearrange("p (t n) -> p t n", t=TILES),
                axis=mybir.AxisListType.X, op=add,
            ))
            emit("v", dve.tensor_scalar_add(seed4_t.ap(), lin4_t.ap(), cst_t.ap()))

            a_last = [None] * TILES
            emit_wait("v", dve, prmU, 16)
            emit_wait("v", dve, ld0, 16)
            mulB_done[0] = mul_range(0, 0, NM)
            tree(0)
            a_last[0] = cttrs(0)
            ldsem = [None, ld1, ld23, ld23]
            ldval = [None, 16, 16, 32]
            for t in range(1, TILES):
                emit_wait("v", dve, ldsem[t], ldval[t])
                mulB_done[t] = mul_range(t, 0, NM)
                tree(t)
                a_last[t] = cttrs(t)
                merge(t - 1, a_last[t - 1])
            merge(TILES - 1, a_last[TILES - 1])
            o2_done[0] = cnt["v"]

        @block.scalar
        def _(act):
            # param loads ride the Activation HWDGE ring, issuing in
            # parallel with the dense loads on the SP ring
            emit_wait("a", act, prm, 16)
            # warmup: trigger the one-time ACT_TABLE_LOAD during the DMA lead-in
            emit("a", act.square(warm_t.ap(), cst_t.ap()))
            for t in range(TILES):
                emit_wait("a", act, vch, mulB_done[t])
                accs = (sqp_t[t], sqn_t[t]) if blocks[0][2] > 0 else (sqn_t[t],)
                for i, (m0, mw, sg) in enumerate(blocks):
                    lo, hi = m0 * N, (m0 + mw) * N
                    emit("a", act.activation(
                        out=y2j.ap()[:, lo:hi], in_=y_t[t].ap()[:, lo:hi],
                        func=Square, accum_out=accs[i].ap(),
                    ))
            assert cnt["a"] == ach_sq_done[-1], (cnt["a"], ach_sq_done)
            # output store from the ACT ring (idle by now), gated on merge3;
            # engine-to-engine sem hop is ~100ns vs 900ns for DMA-completion
            emit_dma("a", act.dma_start(out=out.ap(), in_=o2all.ap()),
                     sts, 16, wait=(vch, o2_done[0]))

        @block.sync
        def _(sync):
            # single ring, few big starts in exact need-order. The DGE
            # needs ~2us to prep a start; back-to-back small starts drain
            # the queues and expose that latency, so keep starts >= 512KB.
            emit_dma("s", sync.dma_start(out=pw_t.ap(), in_=pw.ap()), prm, 16)
            emit_dma("s", sync.dma_start(out=urep_t.ap(), in_=urep.ap()), prmU, 16)
            emit_dma("s", sync.dma_start(
                out=df_t[0].ap(), in_=dense.ap()[0:128, :]), ld0, 16)
            emit_dma("s", sync.dma_start(
                out=df_t[1].ap(), in_=dense.ap()[128:256, :]), ld1, 16)
            emit_dma("s", sync.dma_start(
                out=df_t[2].ap(), in_=dense.ap()[256:384, :]), ld23, 16)
            emit_dma("s", sync.dma_start(
                out=df_t[3].ap(), in_=dense.ap()[384:512, :]), ld23, 16)
            sync.wait_ge(sts, 16)

    nc.compile()
    return nc


def _get_program(key):
    if key not in _CACHE:
        _CACHE[key] = _build_program(*key)
    return _CACHE[key]


def _host_prep(inputs):
    import ml_dtypes

    dense = np.asarray(inputs["dense"], dtype=np.float32)  # [B, N, M]
    v = np.asarray(inputs["v"], dtype=np.float32)          # [N, M]
    Wl = np.asarray(inputs["Wl"], dtype=np.float32).reshape(N)
    Wp = np.asarray(inputs["Wp"], dtype=np.float32).reshape(M)
    bl = float(np.asarray(inputs["bl"], dtype=np.float32).reshape(-1)[0])
    bp = float(np.asarray(inputs["bp"], dtype=np.float32).reshape(-1)[0])

    c = (Wp / (2.0 * P_PAIRS)).astype(np.float32)
    pos = np.where(c >= 0)[0]
    neg = np.where(c < 0)[0]
    idx = np.concatenate([pos, neg])
    K = int(len(pos))

    # m-major, sign-sorted, sqrt|c|-scaled replica of v -> u [64, 32]
    u = (v * np.sqrt(np.abs(c))[None, :]).T[idx]               # [M, N]
    urep = np.ascontiguousarray(np.broadcast_to(
        u.reshape(1, NM).astype(ml_dtypes.bfloat16), (128, NM)))

    # dense repacked m-major + sign-sorted: [B, (m, n)] bf16
    dmm = np.ascontiguousarray(
        dense.transpose(0, 2, 1)[:, idx, :].reshape(B, NM)
    ).astype(ml_dtypes.bfloat16)

    sparse = np.ascontiguousarray(dense[:, :, 0])              # [B, N] f32
    wlrep4 = np.broadcast_to(np.tile(Wl, TILES)[None, :], (128, TILES * N))
    cstv = float(bl + bp)

    in_maps = []
    for i in range(NCORES):
        spdi = (
            sparse[BS * i : BS * (i + 1)]
            .reshape(TILES, 128, N).transpose(1, 0, 2).reshape(128, TILES * N)
        )
        pwi = np.ascontiguousarray(np.concatenate([spdi, wlrep4], axis=1))
        in_maps.append({
            "dense": dmm[BS * i : BS * (i + 1)],
            "urep": urep,
            "pw": pwi,
        })
    return (K, cstv), in_maps


def _gather(res):
    # out[p, t] holds batch row 128*t + p of the core's shard
    outs = []
    for i in range(NCORES):
        arr = np.asarray(res.results[i]["out"], np.float32)  # [128, TILES]
        outs.append(arr.T.reshape(BS))
    return np.concatenate(outs).reshape(B, 1)


def kernel(**inputs) -> np.ndarray:
    from concourse.bass_utils import run_bass_kernel_spmd

    K, in_maps = _host_prep(inputs)
    nc = _get_program(K)
    res = run_bass_kernel_spmd(nc, in_maps, core_ids=list(range(NCORES)))
    return _gather(res)


# revision 19
# speedup vs baseline: 1.0120x; 1.0042x over previous
"""Trainium2 Bass kernel for nn_AFM (attentional factorization machine).

Mathematical reduction (validated against the reference):
  - softmax over a size-1 axis == 1, so the attention MLP is dead code and
    fAtt = mean(fPI, axis=1).
  - FM identity per (b, m): sum_{i<j} x_i x_j = ((sum_i x_i)^2 - sum_i x_i^2)/2
    with x_i = dense[b,i,m] * v[i,m].
  - Sign-split scaling: with c[m] = Wp[m]/(2P), u[n,m] = v[n,m]*sqrt(|c[m]|)
    and y = d*u, the FM term becomes
      sum_m sign(c[m]) * ((sum_n y)^2 - sum_n y^2).
    Host reorders the m axis so all c>=0 columns come first (K of them);
    then sum_m sign*(sum_n y^2) collapses to TWO plain free-axis sums of y^2
    (one per contiguous sign block) - computed on the otherwise-idle
    Activation engine via Square+accum_out, entirely off the DVE.

Layout: m-major bf16. Host repacks dense to [B, (m=64, n=32)] bf16 (halves
HBM traffic; all DVE tensor ops become 2-byte -> 2x DVE rate) and keeps a
separate f32 [B, 32] copy of dense[:, :, 0] for the numerically dominant
linear term. The FM term is ~1e-3 of the output, so bf16 there is safe.

Sharding: pure data parallel, batch 4096 -> 512 rows on each of 8 cores,
4 tiles of 128 rows.

Per-core engine assignment:
  SYNC: all loads on one HWDGE ring as few large starts in exact need-order
        (params -> urep -> t0 -> t1 -> t2+t3). One ring keeps completion
        order == issue order; >=512KB starts hide the ~2us per-start DGE
        prep latency; 4KB row descriptors run the queues at full rate.
        Every load that gates compute has its own semaphore (a start's +16
        completion budget spreads over descriptors, so a shared semaphore
        is only safe at its full total).
  ACT:  per tile two Square+accum_out ops over the sign blocks of y (the
        whole S2 path); a warmup square triggers the one-time
        ACT_TABLE_LOAD early; the final [128, 4] store from the ACT ring
        (engine-to-engine sem hop is ~100ns vs 900ns for DMA completion).
  DVE:  bias memset + the linear term (mul + grouped tensor_reduce over a
        [128, (4t, 32n)] f32 pack of dense[:, :, 0]) run during the DMA
        lead-in; per tile: y = d*u (one full-tile bf16 2x mul), S1 via two
        pairwise bf16 add-tree levels plus one grouped tensor_reduce
        (axis=X) -> [128, 64] f32, two TENSOR_TENSOR_REDUCE ops (+-1,
        seeded with linear+bias) for the signed sum of S1^2, and one
        scalar_tensor_tensor merge per tile folding in the ACT accums.
"""

import numpy as np

B, N, M = 4096, 32, 64
NM = N * M                  # 2048
HALF = NM // 2              # 1024
QTR = NM // 4               # 512
NCORES = 8
BS = B // NCORES            # 512 rows per core
TILES = BS // 128           # 4 tiles of 128 batch rows per core
P_PAIRS = N * (N - 1) // 2  # 496

_CACHE = {}


def _build_program(K, cstv):
    """K = #m columns with c >= 0 (packed first); cstv = bl+bp baked in."""
    from concourse import bacc, mybir
    from concourse.dve_ops import TENSOR_TENSOR_REDUCE as CTTR

    f32 = mybir.dt.float32
    bf16 = mybir.dt.bfloat16
    Square = mybir.ActivationFunctionType.Square
    sub = mybir.AluOpType.subtract
    add = mybir.AluOpType.add

    nc = bacc.Bacc("TRN2", target_bir_lowering=False, debug=False)
    dense = nc.declare_dram_parameter("dense", [BS, NM], bf16, isOutput=False)
    urep = nc.declare_dram_parameter("urep", [128, NM], bf16, isOutput=False)
    pw = nc.declare_dram_parameter("pw", [128, 2 * TILES * N], f32, isOutput=False)
    out = nc.declare_dram_parameter("out", [128, TILES], f32, isOutput=True)

    sb = lambda name, shape, dt: nc.alloc_sbuf_tensor(name, list(shape), dt)

    urep_t = sb("urep_t", [128, NM], bf16)
    pw_t = sb("pw_t", [128, 2 * TILES * N], f32)
    cst_t = sb("cst_t", [128, 1], f32)
    spw_t = sb("spw_t", [128, TILES * N], f32)
    lin4_t = sb("lin4_t", [128, TILES], f32)
    seed4_t = sb("seed4_t", [128, TILES], f32)
    o2all = sb("o2all", [128, TILES], f32)
    warm_t = sb("warm_t", [128, 1], f32)
    y2j = sb("y2j", [128, NM], bf16)       # ACT square junk output
    junk = sb("junk", [128, M], f32)       # DVE CTTR junk output

    df_t, y_t, l0_t, l1_t, s1_t = [], [], [], [], []
    a1_t, a2_t, sqp_t, sqn_t = [], [], [], []
    for t in range(TILES):
        df_t.append(sb(f"df{t}", [128, NM], bf16))
        y_t.append(sb(f"y{t}", [128, NM], bf16))
        l0_t.append(sb(f"l0_{t}", [128, M * 16], bf16))
        l1_t.append(sb(f"l1_{t}", [128, M * 8], bf16))
        s1_t.append(sb(f"s1_{t}", [128, M], f32))
        a1_t.append(sb(f"a1_{t}", [128, 1], f32))
        a2_t.append(sb(f"a2_{t}", [128, 1], f32))
        sqp_t.append(sb(f"sqp_{t}", [128, 1], f32))
        sqn_t.append(sb(f"sqn_{t}", [128, 1], f32))

    cnt = {"v": 0, "a": 0, "s": 0, "g": 0}
    chains = {}

    def emit(e, ins):
        ins._wait_ge(chains[e], cnt[e]).then_inc(chains[e], 1)
        cnt[e] += 1
        return cnt[e]

    def emit_dma(e, ins, sem, inc, wait=None):
        if wait is not None:
            wsem, wval = wait
            ins._wait_ge(wsem, wval)
        else:
            ins._wait_ge(chains[e], cnt[e])
        ins.then_inc(sem, inc)

    def emit_wait(e, eng, sem, val):
        eng.wait_ge(sem, val).then_inc(chains[e], 1)
        cnt[e] += 1

    # sign blocks as (start, width, sign) over the m axis, skipping empties
    blocks = [(0, K, 1.0), (K, M - K, -1.0)]
    blocks = [b for b in blocks if b[1] > 0]

    # ACT chain values after tile t's squares (scalar block is built after
    # the vector block, so predict its chain; asserted below)
    n_sq = len(blocks)
    ach_sq_done = [2 + n_sq * (t + 1) + (t + 1) for t in range(TILES)]

    mulB_done = [0] * TILES
    o2_done = [0]

    # NOTE on DMA semaphores: a dma_start's +16 completion budget is spread
    # over its descriptors, and descriptors of LATER starts on the shared
    # queues can complete before an earlier start's last descriptor. So a
    # shared semaphore only safely gates at its FULL total; every load that
    # gates compute at an intermediate point gets its own semaphore.
    with (
        nc.Block() as block,
        nc.semaphore("vch") as vch,
        nc.semaphore("ach") as ach,
        nc.semaphore("sch") as sch,
        nc.semaphore("ld0") as ld0,
        nc.semaphore("ld1") as ld1,
        nc.semaphore("ld23") as ld23,
        nc.semaphore("prm") as prm,
        nc.semaphore("prmU") as prmU,
        nc.semaphore("sts") as sts,
    ):
        chains.update(v=vch, a=ach, s=sch)

        @block.vector
        def _(dve):
            def mul_range(t, lo, hi):
                return emit("v", dve.tensor_mul(
                    y_t[t].ap()[:, lo:hi], df_t[t].ap()[:, lo:hi],
                    urep_t.ap()[:, lo:hi],
                ))

            def tree(t):
                # n halves 32->16->8 inside each m group (bf16 2x mode),
                # then one grouped reduce [p, 64, 8] -> [p, 64] f32
                src = y_t[t].ap().rearrange("p (m n) -> p m n", m=M)
                d0 = l0_t[t].ap().rearrange("p (m n) -> p m n", m=M)
                emit("v", dve.tensor_add(d0, src[:, :, 0:16], src[:, :, 16:32]))
                d1 = l1_t[t].ap().rearrange("p (m n) -> p m n", m=M)
                emit("v", dve.tensor_add(d1, d0[:, :, 0:8], d0[:, :, 8:16]))
                emit("v", dve.tensor_reduce(
                    s1_t[t].ap(), d1, axis=mybir.AxisListType.X, op=add,
                ))

            def cttrs(t):
                seed = seed4_t.ap()[:, t : t + 1]
                accs = (a1_t[t], a2_t[t])
                for i, (m0, mw, sg) in enumerate(blocks):
                    sl = s1_t[t].ap()[:, m0 : m0 + mw]
                    emit("v", dve._custom_dve(
                        CTTR, out=junk.ap()[:, 0:mw], in0=sl, in1=sl,
                        s0=seed, s1=sg, accum_out=accs[i].ap(),
                    ))
                    seed = accs[i].ap()
                return seed  # [p,1] = seed4 + sum_pos S1^2 - sum_neg S1^2

            def merge(t, a_last):
                emit_wait("v", dve, ach, ach_sq_done[t])
                o2col = o2all.ap()[:, t : t + 1]
                if len(blocks) == 2:
                    # o2 = (sqn - sqp) + a_last
                    emit("v", dve.scalar_tensor_tensor(
                        out=o2col, in0=sqn_t[t].ap(), scalar=sqp_t[t].ap(),
                        in1=a_last, op0=sub, op1=add,
                    ))
                elif blocks[0][2] > 0:  # all positive: o2 = a_last - sqp
                    emit("v", dve.tensor_sub(o2col, a_last, sqp_t[t].ap()))
                else:                   # all negative: o2 = a_last + sqn
                    emit("v", dve.tensor_add(o2col, a_last, sqn_t[t].ap()))

            TN = TILES * N
            # bias constant baked at build time: no DMA, just a memset
            emit("v", dve.memset(cst_t.ap(), cstv))
            # linear term for all 4 tiles runs during the dead DMA wait:
            # lin4[p,t] = sum_n spd[p,t,n]*Wl[n]  (small param lands first)
            emit_wait("v", dve, prm, 16)
            emit("v", dve.tensor_mul(
                spw_t.ap(), pw_t.ap()[:, 0:TN], pw_t.ap()[:, TN : 2 * TN]))
            emit("v", dve.tensor_reduce(
                lin4_t.ap(),
                spw_t.ap().rearrange("p (t n) -> p t n", t=TILES),
                axis=mybir.AxisListType.X, op=add,
            ))
            emit("v", dve.tensor_scalar_add(seed4_t.ap(), lin4_t.ap(), cst_t.ap()))

            a_last = [None] * TILES
            emit_wait("v", dve, prmU, 16)
            emit_wait("v", dve, ld0, 16)
            mulB_done[0] = mul_range(0, 0, NM)
            tree(0)
            a_last[0] = cttrs(0)
            ldsem = [None, ld1, ld23, ld23]
            ldval = [None, 16, 16, 32]
            for t in range(1, TILES):
                emit_wait("v", dve, ldsem[t], ldval[t])
                mulB_done[t] = mul_range(t, 0, NM)
                tree(t)
                a_last[t] = cttrs(t)
                merge(t - 1, a_last[t - 1])
            merge(TILES - 1, a_last[TILES - 1])
            o2_done[0] = cnt["v"]

        @block.scalar
        def _(act):
            # param loads ride the Activation HWDGE ring, issuing in
            # parallel with the dense loads on the SP ring
            emit_wait("a", act, prm, 16)
            # warmup: trigger the one-time ACT_TABLE_LOAD during the DMA lead-in
            emit("a", act.square(warm_t.ap(), cst_t.ap()))
            for t in range(TILES):
                emit_wait("a", act, vch, mulB_done[t])
                accs = (sqp_t[t], sqn_t[t]) if blocks[0][2] > 0 else (sqn_t[t],)
                for i, (m0, mw, sg) in enumerate(blocks):
                    lo, hi = m0 * N, (m0 + mw) * N
                    emit("a", act.activation(
                        out=y2j.ap()[:, lo:hi], in_=y_t[t].ap()[:, lo:hi],
                        func=Square, accum_out=accs[i].ap(),
                    ))
            assert cnt["a"] == ach_sq_done[-1], (cnt["a"], ach_sq_done)
            # output store from the ACT ring (idle by now), gated on merge3;
            # engine-to-engine sem hop is ~100ns vs 900ns for DMA-completion
            emit_dma("a", act.dma_start(out=out.ap(), in_=o2all.ap()),
                     sts, 16, wait=(vch, o2_done[0]))

        @block.sync
        def _(sync):
            # single ring, few big starts in exact need-order. The DGE
            # needs ~2us to prep a start; back-to-back small starts drain
            # the queues and expose that latency, so keep starts >= 512KB.
            emit_dma("s", sync.dma_start(out=pw_t.ap(), in_=pw.ap()), prm, 16)
            emit_dma("s", sync.dma_start(out=urep_t.ap(), in_=urep.ap()), prmU, 16)
            emit_dma("s", sync.dma_start(
                out=df_t[0].ap(), in_=dense.ap()[0:128, :]), ld0, 16)
            emit_dma("s", sync.dma_start(
                out=df_t[1].ap(), in_=dense.ap()[128:256, :]), ld1, 16)
            emit_dma("s", sync.dma_start(
                out=df_t[2].ap(), in_=dense.ap()[256:384, :]), ld23, 16)
            emit_dma("s", sync.dma_start(
                out=df_t[3].ap(), in_=dense.ap()[384:512, :]), ld23, 16)
            sync.wait_ge(sts, 16)

    nc.compile()
    return nc


def _get_program(key):
    if key not in _CACHE:
        _CACHE[key] = _build_program(*key)
    return _CACHE[key]


def _host_prep(inputs):
    import ml_dtypes

    dense = np.asarray(inputs["dense"], dtype=np.float32)  # [B, N, M]
    v = np.asarray(inputs["v"], dtype=np.float32)          # [N, M]
    Wl = np.asarray(inputs["Wl"], dtype=np.float32).reshape(N)
    Wp = np.asarray(inputs["Wp"], dtype=np.float32).reshape(M)
    bl = float(np.asarray(inputs["bl"], dtype=np.float32).reshape(-1)[0])
    bp = float(np.asarray(inputs["bp"], dtype=np.float32).reshape(-1)[0])

    c = (Wp / (2.0 * P_PAIRS)).astype(np.float32)
    pos = np.where(c >= 0)[0]
    neg = np.where(c < 0)[0]
    idx = np.concatenate([pos, neg])
    K = int(len(pos))

    # m-major, sign-sorted, sqrt|c|-scaled replica of v -> u [64, 32]
    u = (v * np.sqrt(np.abs(c))[None, :]).T[idx]               # [M, N]
    urep = np.ascontiguousarray(np.broadcast_to(
        u.reshape(1, NM).astype(ml_dtypes.bfloat16), (128, NM)))

    # dense repacked m-major + sign-sorted: [B, (m, n)] bf16
    dmm = np.ascontiguousarray(
        dense.transpose(0, 2, 1)[:, idx, :].reshape(B, NM)
    ).astype(ml_dtypes.bfloat16)

    sparse = np.ascontiguousarray(dense[:, :, 0])              # [B, N] f32
    wlrep4 = np.broadcast_to(np.tile(Wl, TILES)[None, :], (128, TILES * N))
    cstv = float(bl + bp)

    in_maps = []
    for i in range(NCORES):
        spdi = (
            sparse[BS * i : BS * (i + 1)]
            .reshape(TILES, 128, N).transpose(1, 0, 2).reshape(128, TILES * N)
        )
        pwi = np.ascontiguousarray(np.concatenate([spdi, wlrep4], axis=1))
        in_maps.append({
            "dense": dmm[BS * i : BS * (i + 1)],
            "urep": urep,
            "pw": pwi,
        })
    return (K, cstv), in_maps


def _gather(res):
    # out[p, t] holds batch row 128*t + p of the core's shard
    outs = []
    for i in range(NCORES):
        arr = np.asarray(res.results[i]["out"], np.float32)  # [128, TILES]
        outs.append(arr.T.reshape(BS))
    return np.concatenate(outs).reshape(B, 1)


def kernel(**inputs) -> np.ndarray:
    from concourse.bass_utils import run_bass_kernel_spmd

    K, in_maps = _host_prep(inputs)
    nc = _get_program(K)
    res = run_bass_kernel_spmd(nc, in_maps, core_ids=list(range(NCORES)))
    return _gather(res)
